# revision 13
# baseline (speedup 1.0000x reference)
"""Trainium2 Bass kernel for nn_ExpandingAttention (sparse 27-neighborhood
attention + MLP block).

Strategy: points sorted by flat voxel index, sharded across 8 cores as
contiguous sorted ranges with 512-row halos (per the sharding hint). Pairs
(dst, src-neighbor) are host-packed into 128-row subtiles (2 per 128-dst
tile); raw source features are host-gathered (the halo exchange) and shipped
feature-major.

Device math per pair-subtile is ONE projection matmul producing
[q | score-stat columns | k_raw | v_raw] (LN statistics that are linear in
the input — row means and bias-cross terms — are extra weight columns), a
gpsimd square-accumulate for row sum-of-squares, and a score chain that
evaluates softmax scores directly from RAW k via
    score = rs_k * (q.k_raw + q.bk - mu_k * sum(q))
with rs = exp(-0.5*ln(var)) so the scalar engine only ever uses the
ln/exp activation table in phase A (no activation-table thrash).
The value path scatters [e | e*rs_v | e*mu_v*rs_v | v_raw*(e*rs_v)] through a
one-hot dst matmul; LN bias/mean corrections are applied dst-side, batched.
The MLP runs feature-major: residual x^T is preloaded into PSUM by a
transpose-matmul and W2 accumulates onto it; gelu is the only phase-B2
scalar function. Output is written feature-major and transposed on host.
"""
import os
from contextlib import ExitStack

import numpy as np

import concourse.bass as bass
import concourse.bacc as bacc
import concourse.tile as tile
from concourse import mybir
from concourse.masks import make_identity

# ---------------- problem constants (hardcoded per spec) ----------------
SHAPE = (256, 256, 32)
N = 40000
F = 128
H = 8
D = 16
NCORES = 8
CNT = N // NCORES      # 5000 real points per core
PTS = 5120             # padded dst rows per core (NT tiles of 128)
HALO = 512
NL = HALO + PTS + HALO  # 6144 table rows per core
NT = PTS // 128        # 40 dst tiles
EPS = 1e-5
SUB = 2                # pair subtiles per dst tile (asserted in host prep)
NSUB = NT * SUB        # 80
NG = NSUB // 4         # A-phase groups of 4 subtiles (= 2 tiles)
NB2 = NT // 4          # B2 groups of 4 tiles

# WPROJ column layout
CQ = 0          # q: 0:128
CQBK = 128      # qbk: 128:136
CQSUM = 136     # qsum: 136:144
CKM = 144       # kmean 144, kcross 145, vmean 146, vcross 147
CK = 148        # k_raw: 148:276
CV = 276        # v_raw: 276:404
WPW = 404

# WB (bf16 weights) layout: [wproj | w1 | w2 blocks]
W1OFF = WPW             # 404:916
W2OFF = WPW + 512       # 916:1444
WBW = WPW + 512 + 512

# FPK (f32 consts) layout
FBV = 0        # bv_t broadcast 0:128
FG1 = 128      # g1_t broadcast 128:256
FCMEAN = 256   # [mu_bk, mu_bv] 256:258
FCVAR = 258    # [var(bk)+eps, var(bv)+eps] 258:260
FBM1 = 260     # bm1c [128,4] 260:264
FBM2 = 264     # bm2 per-partition col 264:265
FPW = 272

# rhs / datt column layout
RE = 0      # e8 0:8
RERS = 8    # e*rs_v 8:16
REMRS = 16  # e*mu_v*rs_v 16:24
RAV = 24    # av 24:152
RW = 152

FP32 = mybir.dt.float32
BF16 = mybir.dt.bfloat16

INPUT_SPECS = {
    "featsgT": ([F, NSUB * 128], BF16),  # host-gathered pair-src feats, f-major
    "featsdT": ([F, NSUB * 128], BF16),  # host-gathered pair-DST feats, f-major
    "SELT": ([F, NSUB * 128], BF16),     # one-hot dst scatter, partition=pair
    "featsP": ([F, PTS], FP32),          # residual base (+b1), [p, t*128+f]
    "WB": ([F, WBW], BF16),
    "FPK": ([F, FPW], FP32),
}


# ======================= host-side preparation =======================

def _sort_points(coords):
    X, Y, Z = SHAPE
    fl = (coords[:, 0].astype(np.int64) * (Y * Z)
          + coords[:, 1].astype(np.int64) * Z + coords[:, 2].astype(np.int64))
    return np.argsort(fl, kind="stable")


def _neighbor_table_sorted(cs):
    X, Y, Z = SHAPE
    fl = (cs[:, 0].astype(np.int64) * (Y * Z)
          + cs[:, 1].astype(np.int64) * Z + cs[:, 2].astype(np.int64))
    dense = np.full(X * Y * Z, -1, np.int64)
    dense[fl] = np.arange(N)
    r = np.arange(-1, 2)
    off = np.stack(np.meshgrid(r, r, r, indexing="ij"), -1).reshape(27, 3)
    ncrd = cs[:, None, :].astype(np.int64) + off[None, :, :]
    hi = np.array([X, Y, Z])
    inb = np.all((ncrd >= 0) & (ncrd < hi), axis=-1)
    ncc = np.clip(ncrd, 0, hi - 1)
    nfl = ncc[..., 0] * (Y * Z) + ncc[..., 1] * Z + ncc[..., 2]
    return np.where(inb, dense[nfl], -1)  # [N, 27]


def _build_pairs(idx27):
    """Pair packing -> sel [NC, NSUB, 128, 128] f32 (sel[p, n]=1: pair p of
    the subtile contributes to dst n), src [NC, 128, NSUB] i32 (rows into the
    per-core feats table)."""
    valid = idx27 >= 0
    sel = np.zeros((NCORES, NSUB, 128, 128), np.float32)
    src = np.zeros((NCORES, 128, NSUB), np.int32)
    dst = np.zeros((NCORES, 128, NSUB), np.int32)

    dstg, _slot = np.nonzero(valid)
    srcg = idx27[dstg, _slot]
    core_of = dstg // CNT
    dloc = dstg - core_of * CNT
    tloc = dloc // 128
    n_in_tile = dloc - tloc * 128

    for c in range(NCORES):
        g0 = c * CNT - HALO
        m = core_of == c
        td, tn, ts = tloc[m], n_in_tile[m], srcg[m] - g0
        assert ts.min() >= 0 and ts.max() < NL, "halo too small"
        tile_starts = np.searchsorted(td, np.arange(NT))
        pos = np.arange(len(td)) - tile_starts[td]
        assert pos.max() < SUB * 128, f"pair overflow: {pos.max() + 1}"
        u, p = pos // 128, pos % 128
        sel[c, td * SUB + u, p, tn] = 1.0
        src[c, p, td * SUB + u] = ts
        dst[c, p, td * SUB + u] = HALO + td * 128 + tn
        # dummy dst rows (>= CNT): one self pair each so softmax stays finite
        counts = np.diff(np.concatenate([tile_starts, [len(td)]]))
        for t in range(NT):
            lo = CNT - t * 128
            if lo < 128:
                lo = max(lo, 0)
                for j in range(128 - lo):
                    posd = counts[t] + j
                    ud, pd = posd // 128, posd % 128
                    assert ud < SUB
                    sel[c, t * SUB + ud, pd, lo + j] = 1.0
                    src[c, pd, t * SUB + ud] = HALO + t * 128 + lo + j
                    dst[c, pd, t * SUB + ud] = HALO + t * 128 + lo + j
    return sel, src, dst


def _block_diag(Wk):
    B = np.zeros((F, F), np.float32)
    for h in range(H):
        B[h * D:(h + 1) * D, h * D:(h + 1) * D] = Wk[h].T
    return B


def prepare_in_maps(inputs):
    coords = np.asarray(inputs["coords"])
    feats = np.asarray(inputs["feats"], np.float32)
    Wq = np.asarray(inputs["Wq"], np.float32)
    Wk = np.asarray(inputs["Wk"], np.float32)
    bk = np.asarray(inputs["bk"], np.float32)
    Wv = np.asarray(inputs["Wv"], np.float32)
    bv = np.asarray(inputs["bv"], np.float32)
    g1 = np.asarray(inputs["g1"], np.float32)
    b1 = np.asarray(inputs["b1"], np.float32)
    g2 = np.asarray(inputs["g2"], np.float32)
    b2 = np.asarray(inputs["b2"], np.float32)
    W1 = np.asarray(inputs["W1"], np.float32)
    bm1 = np.asarray(inputs["bm1"], np.float32)
    W2 = np.asarray(inputs["W2"], np.float32)
    bm2 = np.asarray(inputs["bm2"], np.float32)

    order = _sort_points(coords)
    cs, fs = coords[order], feats[order]
    idx27 = _neighbor_table_sorted(cs)
    sel, src, dst = _build_pairs(idx27)

    import ml_dtypes
    bf = lambda a: np.asarray(a, dtype=ml_dtypes.bfloat16)

    scale = float(F) ** -0.5
    wq_f = np.ascontiguousarray((Wq * (g1[:, None] * scale)).T)  # [fi, fo]
    Wkbd = _block_diag(Wk)
    Wvbd = _block_diag(Wv)
    qbk = np.zeros((F, H), np.float32)
    qsum = np.zeros((F, H), np.float32)
    for h in range(H):
        qbk[:, h] = wq_f[:, h * D:(h + 1) * D] @ bk[h * D:(h + 1) * D]
        qsum[:, h] = wq_f[:, h * D:(h + 1) * D].sum(1)
    kmean = Wkbd @ np.full(F, 1.0 / F, np.float32)
    vmean = Wvbd @ np.full(F, 1.0 / F, np.float32)
    mu_bk, mu_bv = bk.mean(), bv.mean()
    kcross = (2.0 / F) * (Wkbd @ bk) - 2.0 * mu_bk * kmean
    vcross = (2.0 / F) * (Wvbd @ bv) - 2.0 * mu_bv * vmean

    wproj = np.zeros((F, WPW), np.float32)
    wproj[:, CQ:CQ + 128] = wq_f
    wproj[:, CQBK:CQBK + 8] = qbk
    wproj[:, CQSUM:CQSUM + 8] = qsum
    wproj[:, CKM] = kmean
    wproj[:, CKM + 1] = kcross
    wproj[:, CKM + 2] = vmean
    wproj[:, CKM + 3] = vcross
    wproj[:, CK:CK + 128] = Wkbd
    wproj[:, CV:CV + 128] = Wvbd

    w1 = np.ascontiguousarray((W1 * g2[None, :]).T)       # [F, 512]
    bm1f = (bm1 + W1 @ b2).astype(np.float32)
    W2T = np.ascontiguousarray(W2.T)                       # [512, 128]

    wb = np.zeros((F, WBW), np.float32)
    wb[:, :WPW] = wproj
    wb[:, W1OFF:W1OFF + 512] = w1
    for jc in range(4):
        wb[:, W2OFF + jc * 128:W2OFF + (jc + 1) * 128] = \
            W2T[jc * 128:(jc + 1) * 128, :]

    fpk = np.zeros((F, FPW), np.float32)
    fpk[:, FBV:FBV + 128] = bv[None, :]
    fpk[:, FG1:FG1 + 128] = g1[None, :]
    fpk[:, FCMEAN] = mu_bk
    fpk[:, FCMEAN + 1] = mu_bv
    fpk[:, FCVAR] = bk.var() + EPS
    fpk[:, FCVAR + 1] = bv.var() + EPS
    fpk[:, FBM1:FBM1 + 4] = bm1f.reshape(4, 128).T
    fpk[:, FBM2] = bm2

    g1_is_one = bool(np.allclose(g1, 1.0))

    in_maps = []
    for c in range(NCORES):
        g0 = c * CNT - HALO
        ftab = np.zeros((NL, F), np.float32)
        lo, hi_ = max(0, g0), min(N, g0 + NL)
        ftab[lo - g0:hi_ - g0] = fs[lo:hi_]
        # host-side neighborhood gather of raw feats rows (halo exchange)
        fg = ftab[src[c].T.reshape(-1)]          # [NSUB*128, F]
        fd = ftab[dst[c].T.reshape(-1)]          # [NSUB*128, F] dst rows
        fp = (ftab[HALO:HALO + PTS] + b1[None, :]).astype(np.float32)
        selT = sel[c].transpose(1, 0, 2).reshape(128, NSUB * 128)
        in_maps.append({
            "featsgT": bf(np.ascontiguousarray(fg.T)),
            "featsdT": bf(np.ascontiguousarray(fd.T)),
            "SELT": bf(np.ascontiguousarray(selT)),
            "featsP": np.ascontiguousarray(fp.reshape(NT, 128, F)
                                           .transpose(1, 0, 2)
                                           .reshape(128, PTS)),
            "WB": bf(wb),
            "FPK": fpk,
        })
    return in_maps, order, g1_is_one


# ======================= device kernel =======================

def _bap(t_ap, offset_delta, ap):
    return bass.AP(tensor=t_ap.tensor, offset=t_ap.offset + offset_delta,
                   ap=ap)


DBG = bool(os.environ.get("KDBG"))


def build_tile_kernel(tc, outs, ins, g1_is_one):
    nc = tc.nc
    out_d = outs["OUT"]
    AL = mybir.AluOpType
    AF = mybir.ActivationFunctionType

    with ExitStack() as ctx:
        sg = ctx.enter_context(tc.tile_pool(name="sg", bufs=1))
        wk = ctx.enter_context(tc.tile_pool(name="wk", bufs=2))
        wk4 = ctx.enter_context(tc.tile_pool(name="wk4", bufs=4))
        pspr = ctx.enter_context(tc.tile_pool(name="pspr", bufs=2,
                                              space="PSUM"))
        psda = ctx.enter_context(tc.tile_pool(name="psda", bufs=2,
                                              space="PSUM"))
        psht = ctx.enter_context(tc.tile_pool(name="psht", bufs=1,
                                              space="PSUM"))
        psh1 = ctx.enter_context(tc.tile_pool(name="psh1", bufs=1,
                                              space="PSUM"))
        pso2 = ctx.enter_context(tc.tile_pool(name="pso2", bufs=1,
                                              space="PSUM"))
        psxt = ctx.enter_context(tc.tile_pool(name="psxt", bufs=1,
                                              space="PSUM"))

        # ---- static tiles ----
        featsgT = sg.tile([F, NSUB * 128], BF16)
        nc.sync.dma_start(out=featsgT[:], in_=ins["featsgT"])
        featsdT = sg.tile([F, NSUB * 128], BF16)
        nc.sync.dma_start(out=featsdT[:], in_=ins["featsdT"])
        selt = sg.tile([F, NSUB * 128], BF16)
        nc.sync.dma_start(out=selt[:], in_=ins["SELT"])
        featsP = sg.tile([F, NT, 128], FP32)
        nc.sync.dma_start(out=featsP[:].rearrange("p t f -> p (t f)"),
                          in_=ins["featsP"])
        wb = sg.tile([F, WBW], BF16)
        nc.sync.dma_start(out=wb[:], in_=ins["WB"])
        fpk = sg.tile([F, FPW], FP32)
        nc.sync.dma_start(out=fpk[:], in_=ins["FPK"])

        id32 = sg.tile([128, 128], FP32)
        make_identity(nc, id32[:])
        id16 = sg.tile([128, 128], BF16)
        make_identity(nc, id16[:])
        zero_t = sg.tile([128, 1], FP32)
        nc.vector.memset(zero_t[:], 0.0)
        eps_t = sg.tile([128, 1], FP32)
        nc.vector.memset(eps_t[:], EPS)

        # persistent accumulators / state
        xt = sg.tile([128, NT, 128], FP32)        # n-major per-tile x
        sqs = sg.tile([128, NSUB, 2], FP32)       # pair sumsq (k, v)
        s2mv = sg.tile([128, NT, 2], FP32)        # per-tile (mean, var) of x
        rs2 = sg.tile([128, NT], FP32)
        negmurs = sg.tile([128, NT], FP32)
        lnu = sg.tile([128, NT], FP32)

        fpk_ap = fpk[:]

        # ------------- phase A + B1: superblocks of 16 subtiles -------------
        SB = 4          # groups per superblock
        NSB = NG // SB  # 5 superblocks
        for sb in range(NSB):
            qsbg = wk.tile([128, 16, 148], BF16, tag="qsbg")
            kvsb = wk.tile([128, 16, 256], BF16, tag="kvsb")
            cmb = wk.tile([128, 16, 24, 16], BF16, tag="cmb")
            rhsg = wk.tile([128, 16, RW], BF16, tag="rhsg")
            red = wk.tile([128, 16, 24], FP32, tag="red")


            for us in range(16):
                iu = 16 * sb + us
                pp = pspr.tile([128, WPW], FP32, tag="pp")
                nc.tensor.matmul(out=pp[:, 0:144],
                                 lhsT=featsdT[:, iu * 128:(iu + 1) * 128],
                                 rhs=wb[:, 0:144], start=True, stop=True)
                nc.tensor.matmul(out=pp[:, 144:WPW],
                                 lhsT=featsgT[:, iu * 128:(iu + 1) * 128],
                                 rhs=wb[:, 144:WPW], start=True, stop=True)
                nc.scalar.activation(out=qsbg[:, us, :], in_=pp[:, 0:148],
                                     func=AF.Copy)
                nc.vector.tensor_copy(out=kvsb[:, us, :], in_=pp[:, CK:CV + 128])
                nc.gpsimd.tensor_tensor(
                    out=cmb[:, us, 0:16, :].rearrange("p a b -> p (a b)"),
                    in0=kvsb[:, us, :], in1=kvsb[:, us, :], op=AL.mult)
                nc.gpsimd.tensor_tensor(
                    out=cmb[:, us, 16:24, :].rearrange("p a b -> p (a b)"),
                    in0=qsbg[:, us, 0:128],
                    in1=kvsb[:, us, 0:128], op=AL.mult)
                nc.vector.tensor_reduce(
                    out=red[:, us, :], in_=cmb[:, us, :, :],
                    axis=mybir.AxisListType.X, op=AL.add)

            # collapse 8-chunk partial sums into sqs
            nc.vector.tensor_reduce(
                out=sqs[:, 16 * sb:16 * sb + 16, :],
                in_=red[:, :, 0:16].rearrange("p u (k c) -> p u k c", k=2),
                axis=mybir.AxisListType.X, op=AL.add)

            # ---- batched stat/score chain over 16 subtiles ----
            mean = qsbg[:, :, 144:148:2]    # kmean, vmean (bf16)
            cross = qsbg[:, :, 145:148:2]
            msq = wk.tile([128, 16, 2], FP32, tag="msq")
            nc.vector.tensor_tensor(out=msq[:], in0=mean, in1=mean,
                                    op=AL.mult)
            varh = wk.tile([128, 16, 2], FP32, tag="varh")
            nc.gpsimd.tensor_scalar(out=varh[:],
                                    in0=sqs[:, 16 * sb:16 * sb + 16, :],
                                    scalar1=1.0 / F, scalar2=None,
                                    op0=AL.mult)
            nc.gpsimd.tensor_tensor(out=varh[:], in0=varh[:], in1=msq[:],
                                    op=AL.subtract)
            nc.gpsimd.tensor_tensor(out=varh[:], in0=varh[:], in1=cross,
                                    op=AL.add)
            cvar_b = _bap(fpk_ap, FCVAR, [fpk_ap.ap[0], [0, 16], [1, 2]])
            nc.gpsimd.tensor_tensor(out=varh[:], in0=varh[:], in1=cvar_b,
                                    op=AL.add)
            lnv = wk.tile([128, 16, 2], FP32, tag="lnv")
            nc.scalar.activation(out=lnv[:], in_=varh[:], func=AF.Ln,
                                 bias=zero_t[:])
            rsb = wk.tile([128, 16, 2], FP32, tag="rsb")
            nc.scalar.activation(out=rsb[:], in_=lnv[:], func=AF.Exp,
                                 bias=zero_t[:], scale=-0.5)
            muh = wk.tile([128, 16, 2], FP32, tag="muh")
            cmean_b = _bap(fpk_ap, FCMEAN, [fpk_ap.ap[0], [0, 16], [1, 2]])
            nc.vector.tensor_tensor(out=muh[:], in0=mean, in1=cmean_b,
                                    op=AL.add)
            mrs = wk.tile([128, 16, 2], FP32, tag="mrs")
            nc.vector.tensor_tensor(out=mrs[:], in0=muh[:], in1=rsb[:],
                                    op=AL.mult)
            sco = wk.tile([128, 16, 8], FP32, tag="sco")
            muk_b = _bap(muh[:], 0, [muh[:].ap[0], [2, 16], [0, 8]])
            nc.gpsimd.tensor_tensor(out=sco[:],
                                    in0=qsbg[:, :, CQSUM:CQSUM + 8],
                                    in1=muk_b, op=AL.mult)
            t2 = wk.tile([128, 16, 8], FP32, tag="t2")
            nc.gpsimd.tensor_tensor(out=t2[:], in0=red[:, :, 16:24],
                                    in1=qsbg[:, :, CQBK:CQBK + 8], op=AL.add)
            nc.gpsimd.tensor_tensor(out=sco[:], in0=t2[:], in1=sco[:],
                                    op=AL.subtract)
            rsk_b = _bap(rsb[:], 0, [rsb[:].ap[0], [2, 16], [0, 8]])
            nc.gpsimd.tensor_tensor(out=sco[:], in0=sco[:], in1=rsk_b,
                                    op=AL.mult)
            nc.scalar.activation(out=rhsg[:, :, RE:RE + 8], in_=sco[:],
                                 func=AF.Exp, bias=zero_t[:])
            rsv_b = _bap(rsb[:], 1, [rsb[:].ap[0], [2, 16], [0, 8]])
            nc.gpsimd.tensor_tensor(out=rhsg[:, :, RERS:RERS + 8],
                                    in0=rhsg[:, :, RE:RE + 8], in1=rsv_b,
                                    op=AL.mult)
            mrsv_b = _bap(mrs[:], 1, [mrs[:].ap[0], [2, 16], [0, 8]])
            nc.gpsimd.tensor_tensor(out=rhsg[:, :, REMRS:REMRS + 8],
                                    in0=rhsg[:, :, RE:RE + 8], in1=mrsv_b,
                                    op=AL.mult)

            for gg in range(SB):
                g = SB * sb + gg
                da = psda.tile([128, 2, RW], FP32, tag="da")
                for u4 in range(4):
                    us = 4 * gg + u4
                    iu = 16 * sb + us
                    r4 = rhsg[:, us, :]
                    ersv_b = _bap(r4, RERS, [r4.ap[0], [1, 8], [0, 16]])
                    nc.gpsimd.tensor_tensor(
                        out=rhsg[:, us, RAV:RAV + 128].rearrange(
                            "p (h d) -> p h d", h=H),
                        in0=kvsb[:, us, 128:256].rearrange(
                            "p (h d) -> p h d", h=H),
                        in1=ersv_b, op=AL.mult)
                    nc.tensor.matmul(out=da[:, u4 // 2, :],
                                     lhsT=selt[:, iu * 128:(iu + 1) * 128],
                                     rhs=rhsg[:, us, :],
                                     start=(u4 % 2 == 0), stop=(u4 % 2 == 1))

                # ---- B1: dst-side corrections for tiles 2g, 2g+1 ----
                rb = wk.tile([128, 2, 8], FP32, tag="rb")
                nc.vector.reciprocal(out=rb[:], in_=da[:, :, RE:RE + 8])
                a2 = wk.tile([128, 2, 8, 16], FP32, tag="a2")
                bv_b = _bap(fpk_ap, FBV, [fpk_ap.ap[0], [0, 2], [16, 8],
                                          [1, 16]])
                s2_b = _bap(da[:], RERS, [da[:].ap[0], [RW, 2], [1, 8],
                                          [0, 16]])
                nc.vector.tensor_tensor(out=a2[:], in0=bv_b, in1=s2_b,
                                        op=AL.mult)
                x1 = wk.tile([128, 2, 128], FP32, tag="x1")
                nc.vector.tensor_tensor(
                    out=x1[:], in0=da[:, :, RAV:RAV + RW - RAV],
                    in1=a2[:].rearrange("p t h d -> p t (h d)"), op=AL.add)
                s3_b = _bap(da[:], REMRS, [da[:].ap[0], [RW, 2], [1, 8],
                                           [0, 16]])
                nc.vector.tensor_tensor(
                    out=x1[:].rearrange("p t (h d) -> p t h d", h=H),
                    in0=x1[:].rearrange("p t (h d) -> p t h d", h=H),
                    in1=s3_b, op=AL.subtract)
                rb_b = _bap(rb[:], 0, [rb[:].ap[0], [8, 2], [1, 8], [0, 16]])
                x3 = wk.tile([128, 2, 128], FP32, tag="x3")
                nc.gpsimd.tensor_tensor(
                    out=x3[:].rearrange("p t (h d) -> p t h d", h=H),
                    in0=x1[:].rearrange("p t (h d) -> p t h d", h=H),
                    in1=rb_b, op=AL.mult)
                if not g1_is_one:
                    g1_b = _bap(fpk_ap, FG1, [fpk_ap.ap[0], [0, 2], [1, 128]])
                    nc.gpsimd.tensor_tensor(out=x3[:], in0=x3[:], in1=g1_b,
                                            op=AL.mult)
                nc.gpsimd.tensor_tensor(out=xt[:, 2 * g:2 * g + 2, :],
                                        in0=x3[:],
                                        in1=featsP[:, 2 * g:2 * g + 2, :],
                                        op=AL.add)
                bns = wk.tile([128, 2, 6], FP32, tag="bns")
                for tt in range(2):
                    t = 2 * g + tt
                    nc.vector.bn_stats(out=bns[:, tt, :], in_=xt[:, t, :])
                    nc.vector.bn_aggr(out=s2mv[:, t, :], in_=bns[:, tt, :])

            # ------- B2 for this superblock: LN2 + MLP on tiles 8sb..8sb+8 ----
            varsl = s2mv[:, 8 * sb:8 * sb + 8, 1:2]
            meansl = s2mv[:, 8 * sb:8 * sb + 8, 0:1]
            lnsl = lnu[:, 8 * sb:8 * sb + 8]
            rssl = rs2[:, 8 * sb:8 * sb + 8]
            nmsl = negmurs[:, 8 * sb:8 * sb + 8]
            nc.scalar.activation(out=lnsl,
                                 in_=varsl.rearrange("p t one -> p (t one)"),
                                 func=AF.Ln, bias=eps_t[:])
            nc.scalar.activation(out=rssl, in_=lnsl, func=AF.Exp,
                                 bias=zero_t[:], scale=-0.5)
            nc.gpsimd.tensor_scalar(out=nmsl,
                                    in0=meansl.rearrange("p t one -> p (t one)"),
                                    scalar1=-1.0, scalar2=None, op0=AL.mult)
            nc.gpsimd.tensor_tensor(out=nmsl, in0=nmsl, in1=rssl, op=AL.mult)

            for g2 in range(2):
                t0 = 8 * sb + 4 * g2
                hng = wk.tile([128, 4, 128], BF16, tag="hng")
                for tt in range(4):
                    t = t0 + tt
                    nc.scalar.activation(out=hng[:, tt, :], in_=xt[:, t, :],
                                         func=AF.Identity,
                                         bias=negmurs[:, t:t + 1],
                                         scale=rs2[:, t:t + 1])
                pht = psht.tile([128, 4, 128], BF16, tag="pht")
                for tt in range(4):
                    nc.tensor.transpose(out=pht[:, tt, :], in_=hng[:, tt, :],
                                        identity=id16[:])
                hnTs = wk.tile([128, 4, 128], BF16, tag="hnTs")
                nc.vector.tensor_copy(out=hnTs[:], in_=pht[:])

                h1s = wk.tile([128, 4, 512], BF16, tag="h1s")
                for jc in range(4):
                    ph = psh1.tile([128, 512], FP32, tag="ph")
                    nc.tensor.matmul(
                        out=ph[:],
                        lhsT=wb[:, W1OFF + jc * 128:W1OFF + (jc + 1) * 128],
                        rhs=hnTs[:].rearrange("p t f -> p (t f)"),
                        start=True, stop=True)
                    nc.scalar.activation(out=h1s[:, jc, :], in_=ph[:],
                                         func=AF.Gelu,
                                         bias=fpk[:, FBM1 + jc:FBM1 + jc + 1],
                                         scale=1.0)
                pxt = psxt.tile([128, 4, 128], FP32, tag="pxt")
                for tt in range(4):
                    t = t0 + tt
                    nc.tensor.transpose(out=pxt[:, tt, :], in_=xt[:, t, :],
                                        identity=id32[:])
                po = pso2.tile([128, 4, 128], FP32, tag="po")
                for jc in range(4):
                    nc.tensor.matmul(
                        out=po[:].rearrange("p t f -> p (t f)"),
                        lhsT=wb[:, W2OFF + jc * 128:W2OFF + (jc + 1) * 128],
                        rhs=h1s[:, jc, :],
                        start=(jc == 0), stop=(jc == 3))
                oT = wk4.tile([128, 4, 128], FP32, tag="oT")
                nc.vector.tensor_scalar_add(out=oT[:], in0=po[:],
                                            scalar1=fpk[:, FBM2:FBM2 + 1])
                nc.vector.tensor_tensor(out=oT[:], in0=oT[:], in1=pxt[:],
                                        op=AL.add)
                nc.sync.dma_start(
                    out=out_d[:, t0 * 128:(t0 + 4) * 128],
                    in_=oT[:].rearrange("p t f -> p (t f)"))


# ======================= public entry point =======================

def _install_ntff_hook():
    try:
        import antenv.axon_hooks  # noqa: F401
        return True
    except ImportError:
        pass
    try:
        import sys
        import types
        if "/root/.axon_site" not in sys.path:
            sys.path.insert(0, "/root/.axon_site")
        from trn_agent_boot.trn_boot import _ntff_profile_via_ctypes
        import antenv
        mod = types.ModuleType("antenv.axon_hooks")
        state = {"h": None}
        mod.set_axon_ntff_profile_hook = lambda h: state.__setitem__("h", h)
        mod.get_axon_ntff_profile_hook = lambda: state["h"]
        sys.modules["antenv.axon_hooks"] = mod
        antenv.axon_hooks = mod
        h = _ntff_profile_via_ctypes("/opt/axon/libaxon_pjrt.so")
        if h is not None:
            mod.set_axon_ntff_profile_hook(h)
        return h is not None
    except Exception as e:  # pragma: no cover
        print(f"ntff hook install failed: {e}")
        return False


def kernel(**inputs):
    from concourse.bass_utils import run_bass_kernel_spmd

    in_maps, order, g1_is_one = prepare_in_maps(inputs)

    nc = bacc.Bacc("TRN2", target_bir_lowering=False, debug=False,
                   num_devices=NCORES)
    ins = {k: nc.dram_tensor(k, shp, dt, kind="ExternalInput").ap()
           for k, (shp, dt) in INPUT_SPECS.items()}
    outs = {"OUT": nc.dram_tensor("OUT", [F, PTS], FP32,
                                  kind="ExternalOutput").ap()}
    if os.environ.get("KDBG"):
        for nm, w, dt in [("DQSB", 4 * 148, BF16), ("DKVS", 4 * 256, BF16),
                          ("DRHS", 4 * RW, BF16), ("DRB", 32, FP32),
                          ("DVARH", 8, FP32), ("DRSB", 8, FP32),
                          ("DSCO", 32, FP32), ("DX3", 256, FP32),
                          ("DXT", NT * 128, FP32), ("DSQS", NSUB * 2, FP32)]:
            outs[nm] = nc.dram_tensor(nm, [128, w], dt,
                                      kind="ExternalOutput").ap()
    with tile.TileContext(nc) as tc:
        build_tile_kernel(tc, outs, ins, g1_is_one)
    nc.compile()

    trace = bool(os.environ.get("BASS_TRACE"))
    if trace:
        trace = _install_ntff_hook()

    res = run_bass_kernel_spmd(
        nc, in_maps, core_ids=list(range(NCORES)), trace=False,
    )

    if trace:
        try:
            res_t = run_bass_kernel_spmd(
                nc, in_maps, core_ids=list(range(NCORES)), trace=True,
            )
            if res_t.exec_time_ns is not None:
                print(f"HW exec time: {res_t.exec_time_ns} ns")
        except Exception as e:
            print(f"traced run failed ({type(e).__name__}); "
                  "falling back to wall-clock estimate")
            res_t = None
        if res_t is None or res_t.exec_time_ns is None:
            import time as _time
            best = None
            for _ in range(3):
                t0 = _time.perf_counter()
                run_bass_kernel_spmd(
                    nc, in_maps, core_ids=list(range(NCORES)), trace=False)
                dt = _time.perf_counter() - t0
                best = dt if best is None else min(best, dt)
            print(f"HW exec time: {int(best * 1e9)} ns")

    sorted_out = np.concatenate(
        [np.asarray(r["OUT"], np.float32).T[:CNT] for r in res.results], 0)
    out = np.empty((N, F), np.float32)
    out[order] = sorted_out
    return out


# revision 15
# speedup vs baseline: 1.0710x; 1.0710x over previous
"""Trainium2 Bass kernel for nn_ExpandingAttention (sparse 27-neighborhood
attention + MLP block).

Strategy: points sorted by flat voxel index, sharded across 8 cores as
contiguous sorted ranges with 512-row halos (per the sharding hint). Pairs
(dst, src-neighbor) are host-packed into 128-row subtiles (2 per 128-dst
tile); raw source features are host-gathered (the halo exchange) and shipped
feature-major.

Device math per pair-subtile is ONE projection matmul producing
[q | score-stat columns | k_raw | v_raw] (LN statistics that are linear in
the input — row means and bias-cross terms — are extra weight columns), a
gpsimd square-accumulate for row sum-of-squares, and a score chain that
evaluates softmax scores directly from RAW k via
    score = rs_k * (q.k_raw + q.bk - mu_k * sum(q))
with rs = exp(-0.5*ln(var)) so the scalar engine only ever uses the
ln/exp activation table in phase A (no activation-table thrash).
The value path scatters [e | e*rs_v | e*mu_v*rs_v | v_raw*(e*rs_v)] through a
one-hot dst matmul; LN bias/mean corrections are applied dst-side, batched.
The MLP runs feature-major: residual x^T is preloaded into PSUM by a
transpose-matmul and W2 accumulates onto it; gelu is the only phase-B2
scalar function. Output is written feature-major and transposed on host.
"""
import os
from contextlib import ExitStack

import numpy as np

import concourse.bass as bass
import concourse.bacc as bacc
import concourse.tile as tile
from concourse import mybir
from concourse.masks import make_identity

# ---------------- problem constants (hardcoded per spec) ----------------
SHAPE = (256, 256, 32)
N = 40000
F = 128
H = 8
D = 16
NCORES = 8
CNT = N // NCORES      # 5000 real points per core
PTS = 5120             # padded dst rows per core (NT tiles of 128)
HALO = 512
NL = HALO + PTS + HALO  # 6144 table rows per core
NT = PTS // 128        # 40 dst tiles
EPS = 1e-5
SUB = 2                # pair subtiles per dst tile (asserted in host prep)
NSUB = NT * SUB        # 80
NG = NSUB // 4         # A-phase groups of 4 subtiles (= 2 tiles)
NB2 = NT // 4          # B2 groups of 4 tiles

# WPROJ column layout
CQ = 0          # q: 0:128
CQBK = 128      # qbk: 128:136
CQSUM = 136     # qsum: 136:144
CKM = 144       # kmean 144, kcross 145, vmean 146, vcross 147
CK = 148        # k_raw: 148:276
CV = 276        # v_raw: 276:404
WPW = 404

# WB (bf16 weights) layout: [wproj | w1 | w2 blocks]
W1OFF = WPW             # 404:916
W2OFF = WPW + 512       # 916:1444
WBW = WPW + 512 + 512

# FPK (f32 consts) layout
FBV = 0        # bv_t broadcast 0:128
FG1 = 128      # g1_t broadcast 128:256
FCMEAN = 256   # [mu_bk, mu_bv] 256:258
FCVAR = 258    # [var(bk)+eps, var(bv)+eps] 258:260
FBM1 = 260     # bm1c [128,4] 260:264
FBM2 = 264     # bm2 per-partition col 264:265
FPW = 272

# rhs / datt column layout
RE = 0      # e8 0:8
RERS = 8    # e*rs_v 8:16
REMRS = 16  # e*mu_v*rs_v 16:24
RAV = 24    # av 24:152
RW = 152

FP32 = mybir.dt.float32
BF16 = mybir.dt.bfloat16

INPUT_SPECS = {
    "featsgT": ([F, NSUB * 128], BF16),  # host-gathered pair-src feats, f-major
    "featsdT": ([F, NSUB * 128], BF16),  # host-gathered pair-DST feats, f-major
    "SELT": ([F, NSUB * 128], BF16),     # one-hot dst scatter, partition=pair
    "featsP": ([F, PTS], FP32),          # residual base (+b1), [p, t*128+f]
    "WB": ([F, WBW], BF16),
    "FPK": ([F, FPW], FP32),
}


# ======================= host-side preparation =======================

def _sort_points(coords):
    X, Y, Z = SHAPE
    fl = (coords[:, 0].astype(np.int64) * (Y * Z)
          + coords[:, 1].astype(np.int64) * Z + coords[:, 2].astype(np.int64))
    return np.argsort(fl, kind="stable")


def _neighbor_table_sorted(cs):
    X, Y, Z = SHAPE
    fl = (cs[:, 0].astype(np.int64) * (Y * Z)
          + cs[:, 1].astype(np.int64) * Z + cs[:, 2].astype(np.int64))
    dense = np.full(X * Y * Z, -1, np.int64)
    dense[fl] = np.arange(N)
    r = np.arange(-1, 2)
    off = np.stack(np.meshgrid(r, r, r, indexing="ij"), -1).reshape(27, 3)
    ncrd = cs[:, None, :].astype(np.int64) + off[None, :, :]
    hi = np.array([X, Y, Z])
    inb = np.all((ncrd >= 0) & (ncrd < hi), axis=-1)
    ncc = np.clip(ncrd, 0, hi - 1)
    nfl = ncc[..., 0] * (Y * Z) + ncc[..., 1] * Z + ncc[..., 2]
    return np.where(inb, dense[nfl], -1)  # [N, 27]


def _build_pairs(idx27):
    """Pair packing -> sel [NC, NSUB, 128, 128] f32 (sel[p, n]=1: pair p of
    the subtile contributes to dst n), src [NC, 128, NSUB] i32 (rows into the
    per-core feats table)."""
    valid = idx27 >= 0
    sel = np.zeros((NCORES, NSUB, 128, 128), np.float32)
    src = np.zeros((NCORES, 128, NSUB), np.int32)
    dst = np.zeros((NCORES, 128, NSUB), np.int32)

    dstg, _slot = np.nonzero(valid)
    srcg = idx27[dstg, _slot]
    core_of = dstg // CNT
    dloc = dstg - core_of * CNT
    tloc = dloc // 128
    n_in_tile = dloc - tloc * 128

    for c in range(NCORES):
        g0 = c * CNT - HALO
        m = core_of == c
        td, tn, ts = tloc[m], n_in_tile[m], srcg[m] - g0
        assert ts.min() >= 0 and ts.max() < NL, "halo too small"
        tile_starts = np.searchsorted(td, np.arange(NT))
        pos = np.arange(len(td)) - tile_starts[td]
        assert pos.max() < SUB * 128, f"pair overflow: {pos.max() + 1}"
        u, p = pos // 128, pos % 128
        sel[c, td * SUB + u, p, tn] = 1.0
        src[c, p, td * SUB + u] = ts
        dst[c, p, td * SUB + u] = HALO + td * 128 + tn
        # dummy dst rows (>= CNT): one self pair each so softmax stays finite
        counts = np.diff(np.concatenate([tile_starts, [len(td)]]))
        for t in range(NT):
            lo = CNT - t * 128
            if lo < 128:
                lo = max(lo, 0)
                for j in range(128 - lo):
                    posd = counts[t] + j
                    ud, pd = posd // 128, posd % 128
                    assert ud < SUB
                    sel[c, t * SUB + ud, pd, lo + j] = 1.0
                    src[c, pd, t * SUB + ud] = HALO + t * 128 + lo + j
                    dst[c, pd, t * SUB + ud] = HALO + t * 128 + lo + j
    return sel, src, dst


def _block_diag(Wk):
    B = np.zeros((F, F), np.float32)
    for h in range(H):
        B[h * D:(h + 1) * D, h * D:(h + 1) * D] = Wk[h].T
    return B


def prepare_in_maps(inputs):
    coords = np.asarray(inputs["coords"])
    feats = np.asarray(inputs["feats"], np.float32)
    Wq = np.asarray(inputs["Wq"], np.float32)
    Wk = np.asarray(inputs["Wk"], np.float32)
    bk = np.asarray(inputs["bk"], np.float32)
    Wv = np.asarray(inputs["Wv"], np.float32)
    bv = np.asarray(inputs["bv"], np.float32)
    g1 = np.asarray(inputs["g1"], np.float32)
    b1 = np.asarray(inputs["b1"], np.float32)
    g2 = np.asarray(inputs["g2"], np.float32)
    b2 = np.asarray(inputs["b2"], np.float32)
    W1 = np.asarray(inputs["W1"], np.float32)
    bm1 = np.asarray(inputs["bm1"], np.float32)
    W2 = np.asarray(inputs["W2"], np.float32)
    bm2 = np.asarray(inputs["bm2"], np.float32)

    order = _sort_points(coords)
    cs, fs = coords[order], feats[order]
    idx27 = _neighbor_table_sorted(cs)
    sel, src, dst = _build_pairs(idx27)

    import ml_dtypes
    bf = lambda a: np.asarray(a, dtype=ml_dtypes.bfloat16)

    scale = float(F) ** -0.5
    wq_f = np.ascontiguousarray((Wq * (g1[:, None] * scale)).T)  # [fi, fo]
    Wkbd = _block_diag(Wk)
    Wvbd = _block_diag(Wv)
    qbk = np.zeros((F, H), np.float32)
    qsum = np.zeros((F, H), np.float32)
    for h in range(H):
        qbk[:, h] = wq_f[:, h * D:(h + 1) * D] @ bk[h * D:(h + 1) * D]
        qsum[:, h] = wq_f[:, h * D:(h + 1) * D].sum(1)
    kmean = Wkbd @ np.full(F, 1.0 / F, np.float32)
    vmean = Wvbd @ np.full(F, 1.0 / F, np.float32)
    mu_bk, mu_bv = bk.mean(), bv.mean()
    kcross = (2.0 / F) * (Wkbd @ bk) - 2.0 * mu_bk * kmean
    vcross = (2.0 / F) * (Wvbd @ bv) - 2.0 * mu_bv * vmean

    wproj = np.zeros((F, WPW), np.float32)
    wproj[:, CQ:CQ + 128] = wq_f
    wproj[:, CQBK:CQBK + 8] = qbk
    wproj[:, CQSUM:CQSUM + 8] = qsum
    wproj[:, CKM] = kmean
    wproj[:, CKM + 1] = kcross
    wproj[:, CKM + 2] = vmean
    wproj[:, CKM + 3] = vcross
    wproj[:, CK:CK + 128] = Wkbd
    wproj[:, CV:CV + 128] = Wvbd

    w1 = np.ascontiguousarray((W1 * g2[None, :]).T)       # [F, 512]
    bm1f = (bm1 + W1 @ b2).astype(np.float32)
    W2T = np.ascontiguousarray(W2.T)                       # [512, 128]

    wb = np.zeros((F, WBW), np.float32)
    wb[:, :WPW] = wproj
    wb[:, W1OFF:W1OFF + 512] = w1
    for jc in range(4):
        wb[:, W2OFF + jc * 128:W2OFF + (jc + 1) * 128] = \
            W2T[jc * 128:(jc + 1) * 128, :]

    fpk = np.zeros((F, FPW), np.float32)
    fpk[:, FBV:FBV + 128] = bv[None, :]
    fpk[:, FG1:FG1 + 128] = g1[None, :]
    fpk[:, FCMEAN] = mu_bk
    fpk[:, FCMEAN + 1] = mu_bv
    fpk[:, FCVAR] = bk.var() + EPS
    fpk[:, FCVAR + 1] = bv.var() + EPS
    fpk[:, FBM1:FBM1 + 4] = bm1f.reshape(4, 128).T
    fpk[:, FBM2] = bm2

    g1_is_one = bool(np.allclose(g1, 1.0))

    in_maps = []
    for c in range(NCORES):
        g0 = c * CNT - HALO
        ftab = np.zeros((NL, F), np.float32)
        lo, hi_ = max(0, g0), min(N, g0 + NL)
        ftab[lo - g0:hi_ - g0] = fs[lo:hi_]
        # host-side neighborhood gather of raw feats rows (halo exchange)
        fg = ftab[src[c].T.reshape(-1)]          # [NSUB*128, F]
        fd = ftab[dst[c].T.reshape(-1)]          # [NSUB*128, F] dst rows
        fp = (ftab[HALO:HALO + PTS] + b1[None, :]).astype(np.float32)
        selT = sel[c].transpose(1, 0, 2).reshape(128, NSUB * 128)
        in_maps.append({
            "featsgT": bf(np.ascontiguousarray(fg.T)),
            "featsdT": bf(np.ascontiguousarray(fd.T)),
            "SELT": bf(np.ascontiguousarray(selT)),
            "featsP": np.ascontiguousarray(fp.reshape(NT, 128, F)
                                           .transpose(1, 0, 2)
                                           .reshape(128, PTS)),
            "WB": bf(wb),
            "FPK": fpk,
        })
    return in_maps, order, g1_is_one


# ======================= device kernel =======================

def _bap(t_ap, offset_delta, ap):
    return bass.AP(tensor=t_ap.tensor, offset=t_ap.offset + offset_delta,
                   ap=ap)


DBG = bool(os.environ.get("KDBG"))


def build_tile_kernel(tc, outs, ins, g1_is_one):
    nc = tc.nc
    out_d = outs["OUT"]
    AL = mybir.AluOpType
    AF = mybir.ActivationFunctionType

    with ExitStack() as ctx:
        sg = ctx.enter_context(tc.tile_pool(name="sg", bufs=1))
        wk = ctx.enter_context(tc.tile_pool(name="wk", bufs=2))
        wk4 = ctx.enter_context(tc.tile_pool(name="wk4", bufs=4))
        pspr = ctx.enter_context(tc.tile_pool(name="pspr", bufs=2,
                                              space="PSUM"))
        psda = ctx.enter_context(tc.tile_pool(name="psda", bufs=2,
                                              space="PSUM"))
        psht = ctx.enter_context(tc.tile_pool(name="psht", bufs=1,
                                              space="PSUM"))
        psh1 = ctx.enter_context(tc.tile_pool(name="psh1", bufs=1,
                                              space="PSUM"))
        pso2 = ctx.enter_context(tc.tile_pool(name="pso2", bufs=1,
                                              space="PSUM"))
        psxt = ctx.enter_context(tc.tile_pool(name="psxt", bufs=1,
                                              space="PSUM"))

        # ---- static tiles ----
        featsgT = sg.tile([F, NSUB * 128], BF16)
        nc.sync.dma_start(out=featsgT[:], in_=ins["featsgT"])
        featsdT = sg.tile([F, NSUB * 128], BF16)
        nc.sync.dma_start(out=featsdT[:], in_=ins["featsdT"])
        selt = sg.tile([F, NSUB * 128], BF16)
        nc.sync.dma_start(out=selt[:], in_=ins["SELT"])
        featsP = sg.tile([F, NT, 128], FP32)
        nc.sync.dma_start(out=featsP[:].rearrange("p t f -> p (t f)"),
                          in_=ins["featsP"])
        wb = sg.tile([F, WBW], BF16)
        nc.sync.dma_start(out=wb[:], in_=ins["WB"])
        fpk = sg.tile([F, FPW], FP32)
        nc.sync.dma_start(out=fpk[:], in_=ins["FPK"])

        id32 = sg.tile([128, 128], FP32)
        make_identity(nc, id32[:])
        id16 = sg.tile([128, 128], BF16)
        make_identity(nc, id16[:])
        zero_t = sg.tile([128, 1], FP32)
        nc.vector.memset(zero_t[:], 0.0)
        eps_t = sg.tile([128, 1], FP32)
        nc.vector.memset(eps_t[:], EPS)

        # persistent accumulators / state
        xt = sg.tile([128, NT, 128], FP32)        # n-major per-tile x
        sqs = sg.tile([128, NSUB, 2], FP32)       # pair sumsq (k, v)
        s2mv = sg.tile([128, NT, 2], FP32)        # per-tile (mean, var) of x
        rs2 = sg.tile([128, NT], FP32)
        negmurs = sg.tile([128, NT], FP32)
        lnu = sg.tile([128, NT], FP32)

        fpk_ap = fpk[:]

        # ------------- phase A + B1: superblocks of 16 subtiles -------------
        SB = 4          # groups per superblock
        NSB = NG // SB  # 5 superblocks
        for sb in range(NSB):
            qsbg = wk.tile([128, 16, 148], BF16, tag="qsbg")
            kvsb = wk.tile([128, 16, 256], BF16, tag="kvsb")
            prodb = wk.tile([128, 16, 128], BF16, tag="prodb")
            junk = wk.tile([128, 16, 256], BF16, tag="junk")
            rhsg = wk.tile([128, 16, RW], BF16, tag="rhsg")
            Rb = wk.tile([128, 16, 8], FP32, tag="Rb")


            for us in range(16):
                iu = 16 * sb + us
                pp = pspr.tile([128, WPW], FP32, tag="pp")
                nc.tensor.matmul(out=pp[:, 0:144],
                                 lhsT=featsdT[:, iu * 128:(iu + 1) * 128],
                                 rhs=wb[:, 0:144], start=True, stop=True)
                nc.tensor.matmul(out=pp[:, 144:WPW],
                                 lhsT=featsgT[:, iu * 128:(iu + 1) * 128],
                                 rhs=wb[:, 144:WPW], start=True, stop=True)
                nc.scalar.activation(out=qsbg[:, us, :], in_=pp[:, 0:148],
                                     func=AF.Copy)
                nc.vector.tensor_copy(out=kvsb[:, us, :], in_=pp[:, CK:CV + 128])
                nc.gpsimd.tensor_tensor(out=junk[:, us, :],
                                        in0=kvsb[:, us, :],
                                        in1=kvsb[:, us, :], op=AL.mult)
                nc.vector.tensor_reduce(
                    out=sqs[:, iu, :],
                    in_=junk[:, us, :].rearrange("p (k f) -> p k f", k=2),
                    axis=mybir.AxisListType.X, op=AL.add)
                nc.gpsimd.tensor_tensor(out=prodb[:, us, :],
                                        in0=qsbg[:, us, 0:128],
                                        in1=kvsb[:, us, 0:128], op=AL.mult)
                nc.vector.tensor_reduce(
                    out=Rb[:, us, :],
                    in_=prodb[:, us, :].rearrange("p (h d) -> p h d", h=H),
                    axis=mybir.AxisListType.X, op=AL.add)

            # ---- batched stat/score chain over 16 subtiles ----
            mean = qsbg[:, :, 144:148:2]    # kmean, vmean (bf16)
            cross = qsbg[:, :, 145:148:2]
            msq = wk.tile([128, 16, 2], FP32, tag="msq")
            nc.vector.tensor_tensor(out=msq[:], in0=mean, in1=mean,
                                    op=AL.mult)
            varh = wk.tile([128, 16, 2], FP32, tag="varh")
            nc.gpsimd.tensor_scalar(out=varh[:],
                                    in0=sqs[:, 16 * sb:16 * sb + 16, :],
                                    scalar1=1.0 / F, scalar2=None,
                                    op0=AL.mult)
            nc.gpsimd.tensor_tensor(out=varh[:], in0=varh[:], in1=msq[:],
                                    op=AL.subtract)
            nc.gpsimd.tensor_tensor(out=varh[:], in0=varh[:], in1=cross,
                                    op=AL.add)
            cvar_b = _bap(fpk_ap, FCVAR, [fpk_ap.ap[0], [0, 16], [1, 2]])
            nc.gpsimd.tensor_tensor(out=varh[:], in0=varh[:], in1=cvar_b,
                                    op=AL.add)
            lnv = wk.tile([128, 16, 2], FP32, tag="lnv")
            nc.scalar.activation(out=lnv[:], in_=varh[:], func=AF.Ln,
                                 bias=zero_t[:])
            if sb > 0:
                pv = sb - 1
                nc.scalar.activation(
                    out=lnu[:, 8 * pv:8 * pv + 8],
                    in_=s2mv[:, 8 * pv:8 * pv + 8, 1:2].rearrange(
                        "p t one -> p (t one)"),
                    func=AF.Ln, bias=eps_t[:])
            rsb = wk.tile([128, 16, 2], FP32, tag="rsb")
            nc.scalar.activation(out=rsb[:], in_=lnv[:], func=AF.Exp,
                                 bias=zero_t[:], scale=-0.5)
            if sb > 0:
                pv = sb - 1
                nc.scalar.activation(out=rs2[:, 8 * pv:8 * pv + 8],
                                     in_=lnu[:, 8 * pv:8 * pv + 8],
                                     func=AF.Exp, bias=zero_t[:], scale=-0.5)
                nc.gpsimd.tensor_scalar(
                    out=negmurs[:, 8 * pv:8 * pv + 8],
                    in0=s2mv[:, 8 * pv:8 * pv + 8, 0:1].rearrange(
                        "p t one -> p (t one)"),
                    scalar1=-1.0, scalar2=None, op0=AL.mult)
                nc.gpsimd.tensor_tensor(out=negmurs[:, 8 * pv:8 * pv + 8],
                                        in0=negmurs[:, 8 * pv:8 * pv + 8],
                                        in1=rs2[:, 8 * pv:8 * pv + 8],
                                        op=AL.mult)
            muh = wk.tile([128, 16, 2], FP32, tag="muh")
            cmean_b = _bap(fpk_ap, FCMEAN, [fpk_ap.ap[0], [0, 16], [1, 2]])
            nc.vector.tensor_tensor(out=muh[:], in0=mean, in1=cmean_b,
                                    op=AL.add)
            mrs = wk.tile([128, 16, 2], FP32, tag="mrs")
            nc.vector.tensor_tensor(out=mrs[:], in0=muh[:], in1=rsb[:],
                                    op=AL.mult)
            sco = wk.tile([128, 16, 8], FP32, tag="sco")
            muk_b = _bap(muh[:], 0, [muh[:].ap[0], [2, 16], [0, 8]])
            nc.gpsimd.tensor_tensor(out=sco[:],
                                    in0=qsbg[:, :, CQSUM:CQSUM + 8],
                                    in1=muk_b, op=AL.mult)
            t2 = wk.tile([128, 16, 8], FP32, tag="t2")
            nc.gpsimd.tensor_tensor(out=t2[:], in0=Rb[:],
                                    in1=qsbg[:, :, CQBK:CQBK + 8], op=AL.add)
            nc.gpsimd.tensor_tensor(out=sco[:], in0=t2[:], in1=sco[:],
                                    op=AL.subtract)
            rsk_b = _bap(rsb[:], 0, [rsb[:].ap[0], [2, 16], [0, 8]])
            nc.gpsimd.tensor_tensor(out=sco[:], in0=sco[:], in1=rsk_b,
                                    op=AL.mult)
            nc.scalar.activation(out=rhsg[:, :, RE:RE + 8], in_=sco[:],
                                 func=AF.Exp, bias=zero_t[:])
            rsv_b = _bap(rsb[:], 1, [rsb[:].ap[0], [2, 16], [0, 8]])
            nc.gpsimd.tensor_tensor(out=rhsg[:, :, RERS:RERS + 8],
                                    in0=rhsg[:, :, RE:RE + 8], in1=rsv_b,
                                    op=AL.mult)
            mrsv_b = _bap(mrs[:], 1, [mrs[:].ap[0], [2, 16], [0, 8]])
            nc.gpsimd.tensor_tensor(out=rhsg[:, :, REMRS:REMRS + 8],
                                    in0=rhsg[:, :, RE:RE + 8], in1=mrsv_b,
                                    op=AL.mult)

            for gg in range(SB):
                g = SB * sb + gg
                da = psda.tile([128, 2, RW], FP32, tag="da")
                for u4 in range(4):
                    us = 4 * gg + u4
                    iu = 16 * sb + us
                    r4 = rhsg[:, us, :]
                    ersv_b = _bap(r4, RERS, [r4.ap[0], [1, 8], [0, 16]])
                    nc.gpsimd.tensor_tensor(
                        out=rhsg[:, us, RAV:RAV + 128].rearrange(
                            "p (h d) -> p h d", h=H),
                        in0=kvsb[:, us, 128:256].rearrange(
                            "p (h d) -> p h d", h=H),
                        in1=ersv_b, op=AL.mult)
                    nc.tensor.matmul(out=da[:, u4 // 2, :],
                                     lhsT=selt[:, iu * 128:(iu + 1) * 128],
                                     rhs=rhsg[:, us, :],
                                     start=(u4 % 2 == 0), stop=(u4 % 2 == 1))

                # ---- B1: dst-side corrections for tiles 2g, 2g+1 ----
                rb = wk.tile([128, 2, 8], FP32, tag="rb")
                nc.vector.reciprocal(out=rb[:], in_=da[:, :, RE:RE + 8])
                a2 = wk.tile([128, 2, 8, 16], FP32, tag="a2")
                bv_b = _bap(fpk_ap, FBV, [fpk_ap.ap[0], [0, 2], [16, 8],
                                          [1, 16]])
                s2_b = _bap(da[:], RERS, [da[:].ap[0], [RW, 2], [1, 8],
                                          [0, 16]])
                nc.vector.tensor_tensor(out=a2[:], in0=bv_b, in1=s2_b,
                                        op=AL.mult)
                x1 = wk.tile([128, 2, 128], FP32, tag="x1")
                nc.vector.tensor_tensor(
                    out=x1[:], in0=da[:, :, RAV:RAV + RW - RAV],
                    in1=a2[:].rearrange("p t h d -> p t (h d)"), op=AL.add)
                s3_b = _bap(da[:], REMRS, [da[:].ap[0], [RW, 2], [1, 8],
                                           [0, 16]])
                nc.vector.tensor_tensor(
                    out=x1[:].rearrange("p t (h d) -> p t h d", h=H),
                    in0=x1[:].rearrange("p t (h d) -> p t h d", h=H),
                    in1=s3_b, op=AL.subtract)
                rb_b = _bap(rb[:], 0, [rb[:].ap[0], [8, 2], [1, 8], [0, 16]])
                x3 = wk.tile([128, 2, 128], FP32, tag="x3")
                nc.gpsimd.tensor_tensor(
                    out=x3[:].rearrange("p t (h d) -> p t h d", h=H),
                    in0=x1[:].rearrange("p t (h d) -> p t h d", h=H),
                    in1=rb_b, op=AL.mult)
                if not g1_is_one:
                    g1_b = _bap(fpk_ap, FG1, [fpk_ap.ap[0], [0, 2], [1, 128]])
                    nc.gpsimd.tensor_tensor(out=x3[:], in0=x3[:], in1=g1_b,
                                            op=AL.mult)
                nc.gpsimd.tensor_tensor(out=xt[:, 2 * g:2 * g + 2, :],
                                        in0=x3[:],
                                        in1=featsP[:, 2 * g:2 * g + 2, :],
                                        op=AL.add)
                bns = wk.tile([128, 2, 6], FP32, tag="bns")
                for tt in range(2):
                    t = 2 * g + tt
                    nc.vector.bn_stats(out=bns[:, tt, :], in_=xt[:, t, :])
                    nc.vector.bn_aggr(out=s2mv[:, t, :], in_=bns[:, tt, :])

            # ------- B2 (MLP) for the PREVIOUS superblock ----
            for g2 in (range(2) if sb > 0 else ()):
                t0 = 8 * (sb - 1) + 4 * g2
                hng = wk.tile([128, 4, 128], BF16, tag="hng")
                for tt in range(4):
                    t = t0 + tt
                    nc.scalar.activation(out=hng[:, tt, :], in_=xt[:, t, :],
                                         func=AF.Identity,
                                         bias=negmurs[:, t:t + 1],
                                         scale=rs2[:, t:t + 1])
                pht = psht.tile([128, 4, 128], BF16, tag="pht")
                for tt in range(4):
                    nc.tensor.transpose(out=pht[:, tt, :], in_=hng[:, tt, :],
                                        identity=id16[:])
                hnTs = wk.tile([128, 4, 128], BF16, tag="hnTs")
                nc.vector.tensor_copy(out=hnTs[:], in_=pht[:])

                h1s = wk.tile([128, 4, 512], BF16, tag="h1s")
                for jc in range(4):
                    ph = psh1.tile([128, 512], FP32, tag="ph")
                    nc.tensor.matmul(
                        out=ph[:],
                        lhsT=wb[:, W1OFF + jc * 128:W1OFF + (jc + 1) * 128],
                        rhs=hnTs[:].rearrange("p t f -> p (t f)"),
                        start=True, stop=True)
                    nc.scalar.activation(out=h1s[:, jc, :], in_=ph[:],
                                         func=AF.Gelu,
                                         bias=fpk[:, FBM1 + jc:FBM1 + jc + 1],
                                         scale=1.0)
                pxt = psxt.tile([128, 4, 128], FP32, tag="pxt")
                for tt in range(4):
                    t = t0 + tt
                    nc.tensor.transpose(out=pxt[:, tt, :], in_=xt[:, t, :],
                                        identity=id32[:])
                po = pso2.tile([128, 4, 128], FP32, tag="po")
                for jc in range(4):
                    nc.tensor.matmul(
                        out=po[:].rearrange("p t f -> p (t f)"),
                        lhsT=wb[:, W2OFF + jc * 128:W2OFF + (jc + 1) * 128],
                        rhs=h1s[:, jc, :],
                        start=(jc == 0), stop=(jc == 3))
                oT = wk4.tile([128, 4, 128], FP32, tag="oT")
                nc.vector.tensor_scalar_add(out=oT[:], in0=po[:],
                                            scalar1=fpk[:, FBM2:FBM2 + 1])
                nc.vector.tensor_tensor(out=oT[:], in0=oT[:], in1=pxt[:],
                                        op=AL.add)
                nc.sync.dma_start(
                    out=out_d[:, t0 * 128:(t0 + 4) * 128],
                    in_=oT[:].rearrange("p t f -> p (t f)"))

        # ---- epilogue: LN2 + MLP for the final superblock ----
        pv = NSB - 1
        nc.scalar.activation(
            out=lnu[:, 8 * pv:8 * pv + 8],
            in_=s2mv[:, 8 * pv:8 * pv + 8, 1:2].rearrange("p t one -> p (t one)"),
            func=AF.Ln, bias=eps_t[:])
        nc.scalar.activation(out=rs2[:, 8 * pv:8 * pv + 8],
                             in_=lnu[:, 8 * pv:8 * pv + 8],
                             func=AF.Exp, bias=zero_t[:], scale=-0.5)
        nc.gpsimd.tensor_scalar(
            out=negmurs[:, 8 * pv:8 * pv + 8],
            in0=s2mv[:, 8 * pv:8 * pv + 8, 0:1].rearrange("p t one -> p (t one)"),
            scalar1=-1.0, scalar2=None, op0=AL.mult)
        nc.gpsimd.tensor_tensor(out=negmurs[:, 8 * pv:8 * pv + 8],
                                in0=negmurs[:, 8 * pv:8 * pv + 8],
                                in1=rs2[:, 8 * pv:8 * pv + 8], op=AL.mult)
        for g2 in range(2):
            t0 = 8 * pv + 4 * g2
            hng = wk.tile([128, 4, 128], BF16, tag="hng")
            for tt in range(4):
                t = t0 + tt
                nc.scalar.activation(out=hng[:, tt, :], in_=xt[:, t, :],
                                     func=AF.Identity,
                                     bias=negmurs[:, t:t + 1],
                                     scale=rs2[:, t:t + 1])
            pht = psht.tile([128, 4, 128], BF16, tag="pht")
            for tt in range(4):
                nc.tensor.transpose(out=pht[:, tt, :], in_=hng[:, tt, :],
                                    identity=id16[:])
            hnTs = wk.tile([128, 4, 128], BF16, tag="hnTs")
            nc.vector.tensor_copy(out=hnTs[:], in_=pht[:])
            h1s = wk.tile([128, 4, 512], BF16, tag="h1s")
            for jc in range(4):
                ph = psh1.tile([128, 512], FP32, tag="ph")
                nc.tensor.matmul(
                    out=ph[:],
                    lhsT=wb[:, W1OFF + jc * 128:W1OFF + (jc + 1) * 128],
                    rhs=hnTs[:].rearrange("p t f -> p (t f)"),
                    start=True, stop=True)
                nc.scalar.activation(out=h1s[:, jc, :], in_=ph[:],
                                     func=AF.Gelu,
                                     bias=fpk[:, FBM1 + jc:FBM1 + jc + 1],
                                     scale=1.0)
            pxt = psxt.tile([128, 4, 128], FP32, tag="pxt")
            for tt in range(4):
                t = t0 + tt
                nc.tensor.transpose(out=pxt[:, tt, :], in_=xt[:, t, :],
                                    identity=id32[:])
            po = pso2.tile([128, 4, 128], FP32, tag="po")
            for jc in range(4):
                nc.tensor.matmul(
                    out=po[:].rearrange("p t f -> p (t f)"),
                    lhsT=wb[:, W2OFF + jc * 128:W2OFF + (jc + 1) * 128],
                    rhs=h1s[:, jc, :],
                    start=(jc == 0), stop=(jc == 3))
            oT = wk4.tile([128, 4, 128], FP32, tag="oT")
            nc.vector.tensor_scalar_add(out=oT[:], in0=po[:],
                                        scalar1=fpk[:, FBM2:FBM2 + 1])
            nc.vector.tensor_tensor(out=oT[:], in0=oT[:], in1=pxt[:],
                                    op=AL.add)
            nc.sync.dma_start(
                out=out_d[:, t0 * 128:(t0 + 4) * 128],
                in_=oT[:].rearrange("p t f -> p (t f)"))


# ======================= public entry point =======================

def _install_ntff_hook():
    try:
        import antenv.axon_hooks  # noqa: F401
        return True
    except ImportError:
        pass
    try:
        import sys
        import types
        if "/root/.axon_site" not in sys.path:
            sys.path.insert(0, "/root/.axon_site")
        from trn_agent_boot.trn_boot import _ntff_profile_via_ctypes
        import antenv
        mod = types.ModuleType("antenv.axon_hooks")
        state = {"h": None}
        mod.set_axon_ntff_profile_hook = lambda h: state.__setitem__("h", h)
        mod.get_axon_ntff_profile_hook = lambda: state["h"]
        sys.modules["antenv.axon_hooks"] = mod
        antenv.axon_hooks = mod
        h = _ntff_profile_via_ctypes("/opt/axon/libaxon_pjrt.so")
        if h is not None:
            mod.set_axon_ntff_profile_hook(h)
        return h is not None
    except Exception as e:  # pragma: no cover
        print(f"ntff hook install failed: {e}")
        return False


def kernel(**inputs):
    from concourse.bass_utils import run_bass_kernel_spmd

    in_maps, order, g1_is_one = prepare_in_maps(inputs)

    nc = bacc.Bacc("TRN2", target_bir_lowering=False, debug=False,
                   num_devices=NCORES)
    ins = {k: nc.dram_tensor(k, shp, dt, kind="ExternalInput").ap()
           for k, (shp, dt) in INPUT_SPECS.items()}
    outs = {"OUT": nc.dram_tensor("OUT", [F, PTS], FP32,
                                  kind="ExternalOutput").ap()}
    if os.environ.get("KDBG"):
        for nm, w, dt in [("DQSB", 4 * 148, BF16), ("DKVS", 4 * 256, BF16),
                          ("DRHS", 4 * RW, BF16), ("DRB", 32, FP32),
                          ("DVARH", 8, FP32), ("DRSB", 8, FP32),
                          ("DSCO", 32, FP32), ("DX3", 256, FP32),
                          ("DXT", NT * 128, FP32), ("DSQS", NSUB * 2, FP32)]:
            outs[nm] = nc.dram_tensor(nm, [128, w], dt,
                                      kind="ExternalOutput").ap()
    with tile.TileContext(nc) as tc:
        build_tile_kernel(tc, outs, ins, g1_is_one)
    nc.compile()

    trace = bool(os.environ.get("BASS_TRACE"))
    if trace:
        trace = _install_ntff_hook()

    res = run_bass_kernel_spmd(
        nc, in_maps, core_ids=list(range(NCORES)), trace=False,
    )

    if trace:
        try:
            res_t = run_bass_kernel_spmd(
                nc, in_maps, core_ids=list(range(NCORES)), trace=True,
            )
            if res_t.exec_time_ns is not None:
                print(f"HW exec time: {res_t.exec_time_ns} ns")
        except Exception as e:
            print(f"traced run failed ({type(e).__name__}); "
                  "falling back to wall-clock estimate")
            res_t = None
        if res_t is None or res_t.exec_time_ns is None:
            import time as _time
            best = None
            for _ in range(3):
                t0 = _time.perf_counter()
                run_bass_kernel_spmd(
                    nc, in_maps, core_ids=list(range(NCORES)), trace=False)
                dt = _time.perf_counter() - t0
                best = dt if best is None else min(best, dt)
            print(f"HW exec time: {int(best * 1e9)} ns")

    sorted_out = np.concatenate(
        [np.asarray(r["OUT"], np.float32).T[:CNT] for r in res.results], 0)
    out = np.empty((N, F), np.float32)
    out[order] = sorted_out
    return out
